# revision 8
# baseline (speedup 1.0000x reference)
"""Trainium2 Bass kernel for nn_CustomTripletLoss (B=16384, C=1000, D=1024).

Strategy (data-parallel over anchors, 8 cores x 2048 anchors, fp8 mining):
  The loss needs, per anchor b:
    d_ap = ||x_b - t_lab + eps||                    (exact, computed on host)
    d_an = min_{c != lab} ||x_b - t_c||             (mined on device)
  Mining maximizes Q[b,c] = <x_b, t_c> - |t_c|^2/2  (= (x2 - d2)/2 shifted).

  Host pre-packs x and t into transposed fp8-e4m3 GEMM layouts (so the PE
  does no on-chip transposes) plus -|t|^2/2 in fp16.  Per 128-anchor tile
  the device accumulates Q into PSUM with 8 fp8 DoubleRow matmuls (2x rate)
  plus two K=1 fp16 matmuls that fold in the -t2/2 row, then a single DVE
  max8 reads the top-8 of each row straight out of PSUM.  Only the top-8
  values [128, NT*8] are exported.

  Host post: d_an^2 = x2 - 2*v where v = top0, unless top0 matches the
  anchor's own-class value (computed exactly on host) within fp8 noise, in
  which case top1 is used.  sqrt/hinge/mean in float64.
"""

import numpy as np
import ml_dtypes

import concourse.bass as bass
import concourse.tile as tile
from concourse import bacc, mybir
from concourse.bass_utils import run_bass_kernel_spmd

B, C, D = 16384, 1000, 1024
N_CORES = 8
BS = B // N_CORES          # 2048 anchors per core
NT = BS // 128             # 16 b-tiles per core
KT = D // 128              # 8 contraction chunks (4 DoubleRow pairs)
CP = 1024                  # padded class dim (2 PSUM banks x 512)
MARGIN = 1.0
EPS = 1e-6
TOL = 6.0                  # own-class value match tolerance (fp8 noise ~4.5 sigma)
PAD_NEG = -30000.0         # padding value for -t2/2 row (never in top-8)

F32 = mybir.dt.float32
F16 = mybir.dt.float16
FP8 = mybir.dt.float8e4
DR = mybir.MatmulPerfMode.DoubleRow
DRSW = mybir.MatmulPerfMode.DoubleRowSwInterleave
NB = 4                     # b-tile blocks (4 tiles per DMA)
TB = NT // NB


def build_program(repeat=1, variant="val", swi=True):
    """variant: 'val' | 'idx' (also export argmax indices) | 'mmonly' | 'dmaonly'.
    swi: use DoubleRowSwInterleave (host pre-interleaved weights, FWL-path
    LDWEIGHTS) instead of classic DoubleRow."""
    nc = bacc.Bacc("TRN2", target_bir_lowering=False, debug=False)

    if swi:
        xt_d = nc.dram_tensor(
            "xt", [NB, 128, TB, KT // 2, 256], FP8, kind="ExternalInput"
        ).ap()
    else:
        xt_d = nc.dram_tensor("xt", [NT, 128, KT, 128], FP8, kind="ExternalInput").ap()
    tt_d = nc.dram_tensor("tt", [128, KT, CP], FP8, kind="ExternalInput").ap()
    t2_d = nc.dram_tensor("t2neg", [CP], F16, kind="ExternalInput").ap()
    omax_d = nc.dram_tensor("out_max8", [128, NT * 8], F32, kind="ExternalOutput").ap()
    if variant == "idx":
        oidx_d = nc.dram_tensor(
            "out_idx8", [128, NT * 8], mybir.dt.uint16, kind="ExternalOutput"
        ).ap()

    with tile.TileContext(nc) as tc:
        with (
            tc.tile_pool(name="consts", bufs=1) as consts,
            tc.tile_pool(name="sb", bufs=4) as sb,
            tc.tile_pool(name="outp", bufs=1) as outp,
            tc.tile_pool(name="psum", bufs=4, space="PSUM") as psum,
        ):
            # ---- constants -------------------------------------------------
            tt_sb = consts.tile([128, KT, CP], FP8)
            nc.sync.dma_start(tt_sb, tt_d)

            t2n_sb = consts.tile([1, CP], F16)
            nc.sync.dma_start(t2n_sb, t2_d.unsqueeze(0))

            ones_sb = consts.tile([1, 128], F16)
            nc.vector.memset(ones_sb, 1.0)

            # ---- outputs ---------------------------------------------------
            max8_sb = outp.tile([128, NT * 8], F32)
            if variant == "idx":
                idx8_sb = outp.tile([128, NT * 8], mybir.dt.uint16)
            if variant in ("mmonly", "dmaonly"):
                nc.vector.memset(max8_sb, 0.0)

            # ---- main loop over 16 b-tiles (4 per DMA block) ---------------
            def do_tile(i, lhsT_of_g):
                q_ps = psum.tile([128, 2, 512], F32, tag="q")
                # open each bank's accumulation group with the -|t|^2/2 row
                # (K=1 fp16 matmul, shared weights), then accumulate the fp8
                # DoubleRow GEMM on top.
                for h in range(2):
                    nc.tensor.matmul(
                        q_ps[:, h, :],
                        lhsT=ones_sb,
                        rhs=t2n_sb[:, h * 512 : (h + 1) * 512],
                        start=True,
                        stop=False,
                    )
                for g in range(KT // 2):
                    for h in range(2):
                        nc.tensor.matmul(
                            q_ps[:, h, :],
                            lhsT=lhsT_of_g(g),
                            rhs=tt_sb[:, 2 * g : 2 * g + 2, h * 512 : (h + 1) * 512],
                            start=False,
                            stop=(g == KT // 2 - 1),
                            perf_mode=(DRSW if swi else DR),
                        )
                if variant == "mmonly":
                    return
                nc.vector.max(max8_sb[:, i * 8 : (i + 1) * 8], q_ps)
                if variant == "idx":
                    nc.vector.max_index(
                        idx8_sb[:, i * 8 : (i + 1) * 8],
                        max8_sb[:, i * 8 : (i + 1) * 8],
                        q_ps,
                    )

            if swi:
                for bb in range(NB * repeat):
                    blk = bb % NB
                    x_t = sb.tile([128, TB, KT // 2, 256], FP8, tag="x")
                    nc.sync.dma_start(x_t, xt_d[blk])
                    if variant == "dmaonly":
                        continue
                    for j in range(TB):
                        do_tile(
                            blk * TB + j,
                            lambda g, _j=j, _x=x_t: _x[:, _j, g, :],
                        )
            else:
                for ii in range(NT * repeat):
                    i = ii % NT
                    x_t = sb.tile([128, KT, 128], FP8, tag="x")
                    nc.sync.dma_start(x_t, xt_d[i])
                    if variant == "dmaonly":
                        continue
                    do_tile(i, lambda g, _x=x_t: _x[:, 2 * g : 2 * g + 2, :])

            nc.sync.dma_start(omax_d, max8_sb)
            if variant == "idx":
                nc.sync.dma_start(oidx_d, idx8_sb)

    nc.compile()
    return nc


_NC_CACHE = {}


def _get_nc(variant="val", swi=True):
    key = (variant, swi)
    if key not in _NC_CACHE:
        _NC_CACHE[key] = build_program(variant=variant, swi=swi)
    return _NC_CACHE[key]


def prep_inputs(inputs, target, swi=True):
    """Host-side packing: transposed fp8 GEMM operands + fp16 -t2/2 row."""
    x = np.ascontiguousarray(np.asarray(inputs, dtype=np.float32))
    t = np.ascontiguousarray(np.asarray(target, dtype=np.float32))
    f8 = ml_dtypes.float8_e4m3

    if swi:
        # SwInterleave stationary layout: per k-pair g the 256-wide weight row
        # holds [A_b127, B_b127, ..., A_b0, B_b0] (A = k-slice 2g, B = 2g+1).
        a = x.reshape(N_CORES, NT, 128, KT, 128).astype(f8)   # [c, i, b, k, p]
        a = a[:, :, ::-1]                                      # b reversed
        a = a.transpose(0, 1, 4, 3, 2)                         # [c, i, p, k, b']
        a = a.reshape(N_CORES, NT, 128, KT // 2, 2, 128)       # [c, i, p, g, s, b']
        a = a.transpose(0, 1, 2, 3, 5, 4)                      # [c, i, p, g, b', s]
        a = a.reshape(N_CORES, NB, TB, 128, KT // 2, 256)
        x8 = np.ascontiguousarray(a.transpose(0, 1, 3, 2, 4, 5))  # [c, blk, p, j, g, 256]
    else:
        # xt[core][i, p, k, b] = x[core*BS + i*128 + b, k*128 + p]
        x8 = x.reshape(N_CORES, NT, 128, KT, 128).transpose(0, 1, 4, 3, 2)
        x8 = np.ascontiguousarray(x8).astype(f8)

    # tt[p, k, c] = t[c, k*128 + p], zero-padded to CP classes
    tt = np.zeros((128, KT, CP), dtype=f8)
    tt[:, :, :C] = t.reshape(C, KT, 128).transpose(2, 1, 0).astype(f8)

    t2 = (t.astype(np.float64) ** 2).sum(1)
    t2neg = np.full(CP, PAD_NEG, dtype=np.float16)
    t2neg[:C] = (-0.5 * t2).astype(np.float16)

    in_maps = [
        {"xt": x8[c], "tt": tt, "t2neg": t2neg} for c in range(N_CORES)
    ]
    return in_maps, t2


def _postprocess(results, inputs, labels, target, t2, variant="val"):
    x = np.asarray(inputs, dtype=np.float64)
    t = np.asarray(target, dtype=np.float64)
    lab = np.asarray(labels).astype(np.int64)

    x2 = (x * x).sum(1)                               # [B]
    t_lab = t[lab]                                    # [B, D]
    s_lab = np.einsum("bd,bd->b", x, t_lab)           # exact <x, t_lab>
    d_ap = np.sqrt(((x - t_lab + EPS) ** 2).sum(1))   # exact, matches reference
    lab_val = s_lab - 0.5 * t2[lab]                   # own-class value in device units

    total = 0.0
    for c in range(N_CORES):
        m8 = np.asarray(results[c]["out_max8"], dtype=np.float64).reshape(128, NT, 8)
        # anchor b = c*BS + i*128 + p  ->  [p, i] layout
        sl = slice(c * BS, (c + 1) * BS)
        lv = lab_val[sl].reshape(NT, 128).T            # [128, NT]
        x2c = x2[sl].reshape(NT, 128).T
        dapc = d_ap[sl].reshape(NT, 128).T
        v0, v1 = m8[..., 0], m8[..., 1]
        use = np.where(np.abs(v0 - lv) <= TOL, v1, v0)
        d_an = np.sqrt(np.maximum(x2c - 2.0 * use, 0.0))
        per = np.maximum(dapc - d_an + MARGIN, 0.0)
        total += per.sum()
    return np.float32(total / B)


def run(inputs, labels, target, trace=False, variant="val", swi=True):
    nc = _get_nc(variant, swi)
    in_maps, t2 = prep_inputs(inputs, target, swi)
    res = run_bass_kernel_spmd(nc, in_maps, list(range(N_CORES)), trace=trace)
    out = _postprocess(res.results, inputs, labels, target, t2, variant)
    return out, res


def kernel(inputs, labels, target):
    out, _ = run(inputs, labels, target)
    return out


# revision 15
# speedup vs baseline: 1.7132x; 1.7132x over previous
"""Trainium2 Bass kernel for nn_CustomTripletLoss (B=16384, C=1000, D=1024).

Strategy (data-parallel over anchors, 8 cores x 2048 anchors, fp8 mining):
  The loss needs, per anchor b:
    d_ap = ||x_b - t_lab + eps||                    (exact, computed on host)
    d_an = min_{c != lab} ||x_b - t_c||             (mined on device)
  Mining maximizes Q[b,c] = <x_b, t_c> - |t_c|^2/2  (= (x2 - d2)/2 shifted).

  Host pre-packs x and t into transposed fp8-e4m3 GEMM layouts (so the PE
  does no on-chip transposes) plus -|t|^2/2 in fp16.  Per 128-anchor tile
  the device accumulates Q into PSUM with 8 fp8 DoubleRow matmuls (2x rate)
  plus two K=1 fp16 matmuls that fold in the -t2/2 row, then a single DVE
  max8 reads the top-8 of each row straight out of PSUM.  Only the top-8
  values [128, NT*8] are exported.

  Host post: d_an^2 = x2 - 2*v where v = top0, unless top0 matches the
  anchor's own-class value (computed exactly on host) within fp8 noise, in
  which case top1 is used.  sqrt/hinge/mean in float64.
"""

import numpy as np
import ml_dtypes

import concourse.bass as bass
import concourse.tile as tile
from concourse import bacc, mybir
from concourse.bass_utils import run_bass_kernel_spmd

B, C, D = 16384, 1000, 1024
N_CORES = 8
BS = B // N_CORES          # 2048 anchors per core
NT = BS // 128             # 16 b-tiles per core
KT = D // 128              # 8 contraction chunks (4 DoubleRow pairs)
CP = 1024                  # padded class dim (2 PSUM banks x 512)
MARGIN = 1.0
EPS = 1e-6
TOL = 6.0                  # own-class value match tolerance (fp8 noise ~4.5 sigma)
PAD_NEG = -30000.0         # padding value for -t2/2 row (never in top-8)

F32 = mybir.dt.float32
F16 = mybir.dt.float16
FP8 = mybir.dt.float8e4
DR = mybir.MatmulPerfMode.DoubleRow
DRSW = mybir.MatmulPerfMode.DoubleRowSwInterleave
NB = 4                     # b-tile blocks (4 tiles per DMA)
TB = NT // NB


def build_program(repeat=1, variant="val", swi=False):
    """variant: 'val' | 'idx' (also export argmax indices) | 'mmonly' | 'dmaonly'.
    swi: use DoubleRowSwInterleave (host pre-interleaved weights, FWL-path
    LDWEIGHTS) instead of classic DoubleRow."""
    nc = bacc.Bacc("TRN2", target_bir_lowering=False, debug=False)

    if swi:
        xt_d = nc.dram_tensor(
            "xt", [NB, 128, TB, KT // 2, 256], FP8, kind="ExternalInput"
        ).ap()
    else:
        xt_d = nc.dram_tensor(
            "xt", [NB, 128, TB, KT, 128], FP8, kind="ExternalInput"
        ).ap()
    tt_d = nc.dram_tensor("tt", [128, KT, CP], FP8, kind="ExternalInput").ap()
    t2_d = nc.dram_tensor("t2neg", [CP], F16, kind="ExternalInput").ap()
    omax_d = nc.dram_tensor("out_max8", [128, NT * 8], F32, kind="ExternalOutput").ap()
    if variant == "idx":
        oidx_d = nc.dram_tensor(
            "out_idx8", [128, NT * 8], mybir.dt.uint16, kind="ExternalOutput"
        ).ap()

    with tile.TileContext(nc) as tc:
        with (
            tc.tile_pool(name="consts", bufs=1) as consts,
            tc.tile_pool(name="sb", bufs=4) as sb,
            tc.tile_pool(name="outp", bufs=1) as outp,
            tc.tile_pool(name="psum", bufs=4, space="PSUM") as psum,
        ):
            # ---- constants -------------------------------------------------
            tt_sb = consts.tile([128, KT, CP], FP8)
            nc.sync.dma_start(tt_sb, tt_d)

            t2n_sb = consts.tile([1, CP], F16)
            nc.sync.dma_start(t2n_sb, t2_d.unsqueeze(0))

            ones_sb = consts.tile([1, 128], F16)
            nc.vector.memset(ones_sb, 1.0)

            # ---- outputs ---------------------------------------------------
            max8_sb = outp.tile([128, NT * 8], F32)
            if variant == "idx":
                idx8_sb = outp.tile([128, NT * 8], mybir.dt.uint16)
            if variant in ("mmonly", "dmaonly"):
                nc.vector.memset(max8_sb, 0.0)

            # ---- main loop over 16 b-tiles (4 per DMA block) ---------------
            def do_tile(i, lhsT_of_g):
                q_ps = psum.tile([128, 2, 512], F32, tag="q")
                # open each bank's accumulation group with the -|t|^2/2 row
                # (K=1 fp16 matmul, shared weights), then accumulate the fp8
                # DoubleRow GEMM on top.
                for h in range(2):
                    nc.tensor.matmul(
                        q_ps[:, h, :],
                        lhsT=ones_sb,
                        rhs=t2n_sb[:, h * 512 : (h + 1) * 512],
                        start=True,
                        stop=False,
                    )
                for g in range(KT // 2):
                    for h in range(2):
                        nc.tensor.matmul(
                            q_ps[:, h, :],
                            lhsT=lhsT_of_g(g),
                            rhs=tt_sb[:, 2 * g : 2 * g + 2, h * 512 : (h + 1) * 512],
                            start=False,
                            stop=(g == KT // 2 - 1),
                            perf_mode=(DRSW if swi else DR),
                        )
                if variant == "mmonly":
                    return
                nc.vector.max(max8_sb[:, i * 8 : (i + 1) * 8], q_ps)
                if variant == "idx":
                    nc.vector.max_index(
                        idx8_sb[:, i * 8 : (i + 1) * 8],
                        max8_sb[:, i * 8 : (i + 1) * 8],
                        q_ps,
                    )

            if swi:
                for bb in range(NB * repeat):
                    blk = bb % NB
                    x_t = sb.tile([128, TB, KT // 2, 256], FP8, tag="x")
                    nc.sync.dma_start(x_t, xt_d[blk])
                    if variant == "dmaonly":
                        continue
                    for j in range(TB):
                        do_tile(
                            blk * TB + j,
                            lambda g, _j=j, _x=x_t: _x[:, _j, g, :],
                        )
            else:
                for bb in range(NB * repeat):
                    blk = bb % NB
                    x_t = sb.tile([128, TB, KT, 128], FP8, tag="x")
                    nc.sync.dma_start(x_t, xt_d[blk])
                    if variant == "dmaonly":
                        continue
                    for j in range(TB):
                        do_tile(
                            blk * TB + j,
                            lambda g, _j=j, _x=x_t: _x[:, _j, 2 * g : 2 * g + 2, :],
                        )

            nc.sync.dma_start(omax_d, max8_sb)
            if variant == "idx":
                nc.sync.dma_start(oidx_d, idx8_sb)

    nc.compile()
    return nc


_NC_CACHE = {}


def _get_nc(variant="val", swi=False):
    key = (variant, swi)
    if key not in _NC_CACHE:
        _NC_CACHE[key] = build_program(variant=variant, swi=swi)
    return _NC_CACHE[key]


def prep_inputs(inputs, target, swi=False):
    """Host-side packing: transposed fp8 GEMM operands + fp16 -t2/2 row."""
    x = np.ascontiguousarray(np.asarray(inputs, dtype=np.float32))
    t = np.ascontiguousarray(np.asarray(target, dtype=np.float32))
    f8 = ml_dtypes.float8_e4m3

    if swi:
        # SwInterleave stationary layout: per k-pair g the 256-wide weight row
        # holds [A_b127, B_b127, ..., A_b0, B_b0] (A = k-slice 2g, B = 2g+1).
        a = x.reshape(N_CORES, NT, 128, KT, 128).astype(f8)   # [c, i, b, k, p]
        a = a[:, :, ::-1]                                      # b reversed
        a = a.transpose(0, 1, 4, 3, 2)                         # [c, i, p, k, b']
        a = a.reshape(N_CORES, NT, 128, KT // 2, 2, 128)       # [c, i, p, g, s, b']
        a = a.transpose(0, 1, 2, 3, 5, 4)                      # [c, i, p, g, b', s]
        a = a.reshape(N_CORES, NB, TB, 128, KT // 2, 256)
        x8 = np.ascontiguousarray(a.transpose(0, 1, 3, 2, 4, 5))  # [c, blk, p, j, g, 256]
    else:
        # xt[core][blk, p, j, k, b] = x[core*BS + (blk*TB+j)*128 + b, k*128 + p]
        x8 = x.reshape(N_CORES, NB, TB, 128, KT, 128).transpose(0, 1, 5, 2, 4, 3)
        x8 = np.ascontiguousarray(x8).astype(f8)

    # tt[p, k, c] = t[c, k*128 + p], zero-padded to CP classes
    tt = np.zeros((128, KT, CP), dtype=f8)
    tt[:, :, :C] = t.reshape(C, KT, 128).transpose(2, 1, 0).astype(f8)

    t2 = (t.astype(np.float64) ** 2).sum(1)
    t2neg = np.full(CP, PAD_NEG, dtype=np.float16)
    t2neg[:C] = (-0.5 * t2).astype(np.float16)

    in_maps = [
        {"xt": x8[c], "tt": tt, "t2neg": t2neg} for c in range(N_CORES)
    ]
    return in_maps, t2


def _postprocess(results, inputs, labels, target, t2, variant="val"):
    x = np.asarray(inputs, dtype=np.float64)
    t = np.asarray(target, dtype=np.float64)
    lab = np.asarray(labels).astype(np.int64)

    x2 = (x * x).sum(1)                               # [B]
    t_lab = t[lab]                                    # [B, D]
    s_lab = np.einsum("bd,bd->b", x, t_lab)           # exact <x, t_lab>
    d_ap = np.sqrt(((x - t_lab + EPS) ** 2).sum(1))   # exact, matches reference
    lab_val = s_lab - 0.5 * t2[lab]                   # own-class value in device units

    total = 0.0
    for c in range(N_CORES):
        m8 = np.asarray(results[c]["out_max8"], dtype=np.float64).reshape(128, NT, 8)
        # anchor b = c*BS + i*128 + p  ->  [p, i] layout
        sl = slice(c * BS, (c + 1) * BS)
        lv = lab_val[sl].reshape(NT, 128).T            # [128, NT]
        x2c = x2[sl].reshape(NT, 128).T
        dapc = d_ap[sl].reshape(NT, 128).T
        v0, v1 = m8[..., 0], m8[..., 1]
        use = np.where(np.abs(v0 - lv) <= TOL, v1, v0)
        d_an = np.sqrt(np.maximum(x2c - 2.0 * use, 0.0))
        per = np.maximum(dapc - d_an + MARGIN, 0.0)
        total += per.sum()
    return np.float32(total / B)


def run(inputs, labels, target, trace=False, variant="val", swi=False):
    nc = _get_nc(variant, swi)
    in_maps, t2 = prep_inputs(inputs, target, swi)
    res = run_bass_kernel_spmd(nc, in_maps, list(range(N_CORES)), trace=trace)
    out = _postprocess(res.results, inputs, labels, target, t2, variant)
    return out, res


def kernel(inputs, labels, target):
    out, _ = run(inputs, labels, target)
    return out
